# revision 37
# baseline (speedup 1.0000x reference)
"""Trainium2 Bass kernel for a dense-transformer attention block.

Module: y = o_proj(causal_sdpa(rope(q_proj(x)), rope(k_proj(x)), v_proj(x)))
Shapes: x [2, 2048, 2048], 32 q heads / 8 kv heads, head_dim 64, fp32 I/O.

Sharding (8 NeuronCores): 2-way data parallel over batch x 4-way tensor
parallel over heads. Core c handles batch c//4 and head group c%4
(8 q heads, 2 kv heads). Each core produces a partial [2048, 2048]
output (its heads' slice of o_proj); the host sums the 4 partials per
batch.

On-device layout (v2 — transpose-free):
- q/k projections run weight-stationary (lhsT = W chunk, moving = xT), so
  they produce qT/kT [feat, seq] directly — no PE transposes. RoPE is
  applied at PSUM eviction in this transposed layout: rotate-half
  partners are +-32 partitions away, handled by four [32,512] shift-mult
  ops against a sign-folded sin table, plus one cos-mult and one add.
- v runs x-stationary (natural [seq, feat] layout) and is packed into
  vext = [V | ones] 128-wide stationary tiles.
- Scores are computed transposed (ST = K Q^T per 128x512 block,
  64-partition contraction, no zero padding), exp on eviction with
  scale=1/8 (no max subtraction; |S/8| < ~10 for this distribution).
- O matmul is V-stationary: out = vext^T @ P gives OT [d, q] on
  partitions 0:64 and the softmax denominator replicated on partitions
  64:128 (the 64 ones-columns broadcast it). Normalization is then a
  pure elementwise reciprocal+multiply writing oT [feat, seq] tiles,
  which feed o_proj's stationary operand directly — no O transposes.
- k is written duplicated into both 64-partition halves so the S matmul
  stationary base partition always matches the q head's base partition.
- Work is interleaved in rounds over 512-wide seq chunks: projection
  round B(r) then attention round A(r) (which needs only k/v tiles
  0..4r+3), so exp on the ACT engine overlaps PE work from early on and
  input DMA is pipelined with the first matmuls.
"""

import os
import sys
import types

import numpy as np

sys.path.insert(0, "/opt/trn_rl_repo")

import concourse.bacc as bacc  # noqa: E402
import concourse.bass as bass  # noqa: E402
import concourse.tile as tile  # noqa: E402
from concourse import mybir  # noqa: E402
from concourse.bass_utils import run_bass_kernel_spmd  # noqa: E402

try:
    import ml_dtypes
    BF16 = ml_dtypes.bfloat16
except ImportError:  # pragma: no cover
    BF16 = np.dtype("bfloat16")

HIDDEN = 2048
SEQ = 2048
BATCH = 2
N_HEADS = 32
N_KV_HEADS = 8
HEAD_DIM = 64
ROPE_THETA = 10000.0

N_CORES = 8
TP = 4                      # head-parallel ways
QH = N_HEADS // TP          # 8 q heads per core
KVH = N_KV_HEADS // TP      # 2 kv heads per core
KT = HIDDEN // 128          # 16 contraction tiles
TT = SEQ // 128             # 16 seq tiles
NR = 4                      # rounds (512-wide seq chunks)
F_O = QH * HEAD_DIM         # 512

FP32 = mybir.dt.float32
BF16_DT = mybir.dt.bfloat16
F8_DT = mybir.dt.float8e4
DR = mybir.MatmulPerfMode.DoubleRow
KB = 8                      # fp8 DoubleRow contraction blocks (256 wide)
W_SCALE = 64.0              # host upscale of W into fp8 normal range

try:
    F8NP = ml_dtypes.float8_e4m3fn
except Exception:  # pragma: no cover
    F8NP = None


def _build_nc():
    nc = bacc.Bacc("TRN2", target_bir_lowering=False, debug=False)

    xT = nc.dram_tensor("xT", [HIDDEN, SEQ], BF16_DT, kind="ExternalInput")
    wqkv = nc.dram_tensor("wqkv", [HIDDEN, 768], BF16_DT, kind="ExternalInput")
    wo = nc.dram_tensor("wo", [F_O, HIDDEN], BF16_DT, kind="ExternalInput")
    cosT = nc.dram_tensor("cosT", [128, SEQ], BF16_DT, kind="ExternalInput")
    msinT = nc.dram_tensor("msinT", [128, SEQ], BF16_DT, kind="ExternalInput")
    maskt = nc.dram_tensor("maskt", [128, 128], BF16_DT, kind="ExternalInput")
    out = nc.dram_tensor("out", [SEQ, HIDDEN], FP32, kind="ExternalOutput")

    with tile.TileContext(nc) as tc:
        _emit(nc, tc, xT, wqkv, wo, cosT, msinT, maskt, out)
    nc.compile()
    return nc


def _emit(nc, tc, xT, wqkv, wo, cosT, msinT, maskt, out, dumps=None):
    from contextlib import ExitStack
    ctx = ExitStack()
    Exp = mybir.ActivationFunctionType.Exp
    mult = mybir.AluOpType.mult

    const = ctx.enter_context(tc.tile_pool(name="const", bufs=1))
    persist = ctx.enter_context(tc.tile_pool(name="persist", bufs=1))
    big = ctx.enter_context(tc.tile_pool(name="big", bufs=1))
    bwork = ctx.enter_context(tc.tile_pool(name="bwork", bufs=4))
    att = ctx.enter_context(tc.tile_pool(name="att", bufs=1))
    fwork = ctx.enter_context(tc.tile_pool(name="fwork", bufs=3))
    psS = ctx.enter_context(tc.tile_pool(name="psS", bufs=2, space="PSUM"))
    psO = ctx.enter_context(tc.tile_pool(name="psO", bufs=2, space="PSUM"))
    ps512 = ctx.enter_context(tc.tile_pool(name="ps512", bufs=2, space="PSUM"))

    # ---- constants / persistent buffers ----
    mask_sb = const.tile([128, 128], BF16_DT)
    cos_sb = const.tile([128, SEQ], BF16_DT)
    msin_sb = const.tile([128, SEQ], BF16_DT)
    wo_sb = const.tile([128, 4, HIDDEN], BF16_DT)

    # qT: head h lives at partitions 64*(h%2), pair index h//2.
    # kT: kv head j duplicated on partitions 0:64 AND 64:128 so the S
    # matmul's stationary base always matches the q head's base partition.
    # vext: two variants per (seq tile, kv head): [V | ones] for even
    # heads, [ones | V] for odd heads, so OT lands on the head's oT
    # partitions and the ones block broadcasts the softmax denominator to
    # the other 64 partitions.
    # oT: o_proj feature chunk fh holds heads 2fh (parts 0:64), 2fh+1.
    qT_sb = persist.tile([128, NR, SEQ], BF16_DT, name="qT")
    kT_sb = persist.tile([128, KVH, SEQ], BF16_DT, name="kT")
    vext_sb = persist.tile([128, TT, KVH, 192], BF16_DT, name="vext")
    oT_sb = persist.tile([128, 4, SEQ], BF16_DT, name="oT")
    xT_sb = big.tile([128, KT, SEQ], BF16_DT)
    w_sb = big.tile([128, KT, 768], BF16_DT)

    nc.gpsimd.memset(vext_sb[:, :, :, 0:64], 1.0)
    nc.gpsimd.memset(vext_sb[:, :, :, 128:192], 1.0)

    # ---- input DMA: consts, then (w, x) per k-slice for round 0, then
    # the remaining x seq-chunks ----
    nc.sync.dma_start(out=mask_sb[:], in_=maskt[:])
    nc.sync.dma_start(out=cos_sb[:], in_=cosT[:])
    nc.sync.dma_start(out=msin_sb[:], in_=msinT[:])
    xT_r = xT[:].rearrange("(k p) t -> p k t", p=128)
    w_r = wqkv[:].rearrange("(k p) f -> p k f", p=128)
    for k in range(KT):
        nc.sync.dma_start(out=w_sb[:, k, :], in_=w_r[:, k, :])
        nc.sync.dma_start(out=xT_sb[:, k, 0:512], in_=xT_r[:, k, 0:512])
    nc.sync.dma_start(out=wo_sb[:], in_=wo[:].rearrange("(k p) d -> p k d", p=128))
    for k in range(KT):
        nc.sync.dma_start(out=xT_sb[:, k, 512:SEQ], in_=xT_r[:, k, 512:SEQ])

    def rope_evict(ps, dst0, rsl):
        """RoPE at PSUM eviction, transposed layout. ps is [128, 512] with
        two heads stacked (64 partitions each). The rotate-half partner
        shuffle (+-32 partitions) runs as SBUF->SBUF DMAs since DVE lanes
        cannot cross partitions; the multiplies are then full-width and
        partition-aligned."""
        src = bwork.tile([128, 512], BF16_DT, tag="src", name="src")
        nc.scalar.copy(src[:], ps)
        shf = bwork.tile([128, 512], BF16_DT, tag="shf", name="shf")
        for od, os_ in ((0, 32), (32, 0), (64, 96), (96, 64)):
            nc.gpsimd.dma_start(out=shf[od:od + 32, :],
                                in_=src[os_:os_ + 32, :])
        nc.vector.tensor_tensor(dst0, src[:], cos_sb[:, rsl], op=mult)
        tmp = bwork.tile([128, 512], BF16_DT, tag="tmp", name="tmp")
        nc.vector.tensor_tensor(tmp[:], shf[:], msin_sb[:, rsl], op=mult)
        nc.vector.tensor_add(dst0, dst0, tmp[:])

    # ---- job generators: B(r) projection jobs, A(r) head jobs,
    # P(r) o_proj jobs. Emission interleaves streams job-by-job so the PE
    # queue always has independent work to fill dependency bubbles (keeps
    # HAM from re-throttling on >3us gaps). ----

    def b_jobs(r):
        rsl = bass.ds(r * 512, 512)

        def kchunk():
            kps = ps512.tile([128, 512], FP32, tag="c", name="kps")
            for k in range(KT):
                nc.tensor.matmul(kps[:], w_sb[:, k, 512:640],
                                 xT_sb[:, k, rsl],
                                 start=(k == 0), stop=(k == KT - 1))
            # RoPE into a temp, then duplicate each kv head to both halves
            # (aligned halves via DVE, cross-partition halves via DMA).
            kt_tmp = bwork.tile([128, 512], BF16_DT, tag="ktt", name="kt_tmp")
            rope_evict(kps[:], kt_tmp[:], rsl)
            nc.vector.tensor_copy(kT_sb[0:64, 0, rsl], kt_tmp[0:64, :])
            nc.gpsimd.dma_start(out=kT_sb[64:128, 0, rsl],
                                in_=kt_tmp[0:64, :])
            nc.gpsimd.dma_start(out=kT_sb[0:64, 1, rsl],
                                in_=kt_tmp[64:128, :])
            nc.vector.tensor_copy(kT_sb[64:128, 1, rsl], kt_tmp[64:128, :])
        yield kchunk

        def vtile(t):
            tsl = bass.ds(t * 128, 128)
            vps = ps512.tile([128, 512], FP32, tag="c", name="vps")
            for k in range(KT):
                nc.tensor.matmul(vps[:, 0:128], xT_sb[:, k, tsl],
                                 w_sb[:, k, 640:768],
                                 start=(k == 0), stop=(k == KT - 1))
            vnat = vps[:, 0:128].rearrange("p (j d) -> p j d", j=KVH)
            nc.vector.tensor_copy(vext_sb[:, t, :, 64:128], vnat)

        def qchunk(fc):
            qps = ps512.tile([128, 512], FP32, tag="c", name="qps")
            for k in range(KT):
                nc.tensor.matmul(qps[:], w_sb[:, k, bass.ds(fc * 128, 128)],
                                 xT_sb[:, k, rsl],
                                 start=(k == 0), stop=(k == KT - 1))
            rope_evict(qps[:], qT_sb[:, fc, rsl], rsl)

        # q chunks before v tiles: the q/k RoPE eviction chains (DVE +
        # shuffle DMA) are long, the v evictions short, so this order
        # has all chains drained by the time A(r) starts.
        for fc in range(4):
            yield (lambda fc=fc: qchunk(fc))
        for t in range(4 * r, 4 * r + 4):
            yield (lambda t=t: vtile(t))

    def a_jobs(r):
        rsl = bass.ds(r * 512, 512)
        n_ik = 4 * r + 4

        def headpair(i):
            # heads h0=2i (partitions 0:64) and h1=2i+1 (64:128). Their S
            # matmuls contract only 64 rows each, at row groups 0/64 —
            # the PE runs them concurrently (per-subarray row tiling), so
            # the pair's score block costs one matmul's wall time.
            jv = (2 * i) // (QH // KVH)
            Ops = [psO.tile([128, 512], FP32, tag="O", name=f"Ops{r}_{i}{m}")
                   for m in range(2)]
            # O matmuls trail the S/exp stream by two iterations so the
            # exp (ACT) + mask (gpsimd) latency is hidden behind the next
            # two score matmuls instead of stalling the PE every step.
            pend = []  # [(ik, j0, p_sb), ...] awaiting O matmuls
            for ik in range(n_ik):
                j0 = max(0, ik - 4 * r)
                lsl = bass.ds(j0 * 128, 512 - j0 * 128)
                qsl = bass.ds(r * 512 + j0 * 128, 512 - j0 * 128)
                stp = psS.tile([128, 2, 512], FP32, tag="st", name="stp")
                for m, hp in ((0, 0), (1, 64)):
                    nc.tensor.matmul(stp[:, m, lsl],
                                     kT_sb[hp:hp + 64, jv, bass.ts(ik, 128)],
                                     qT_sb[hp:hp + 64, i, qsl],
                                     start=True, stop=True)
                p_sb = att.tile([128, 2, 512], BF16_DT, tag="p", bufs=4,
                                name="p_sb")
                nc.scalar.activation(p_sb[:, :, lsl], stp[:, :, lsl],
                                     Exp, scale=0.125)
                if ik >= 4 * r:  # diagonal tile: apply causal mask
                    for m in range(2):
                        nc.gpsimd.tensor_mul(
                            p_sb[:, m, bass.ts(j0, 128)],
                            p_sb[:, m, bass.ts(j0, 128)], mask_sb[:])
                if len(pend) >= 2:
                    _o_mms(nc, Ops, vext_sb, jv, n_ik, *pend.pop(0))
                pend.append((ik, j0, p_sb))
            for args in pend:
                _o_mms(nc, Ops, vext_sb, jv, n_ik, *args)

            # Evict both O accumulators to SBUF right away so the PSUM
            # banks free for the next pair; normalization then runs off
            # the critical path. OT sits at partitions hp:hp+64, the
            # replicated denominator at the other half; a small
            # SBUF->SBUF DMA moves the reciprocal to OT's partitions so
            # the normalizing multiply is partition-aligned.
            direct = (r == NR - 1 and i == QH // 2 - 1)
            for m, hp in ((0, 0), (1, 64)):
                dp = 64 - hp
                if direct:
                    osb = Ops[m]
                else:
                    osb = att.tile([128, 512], FP32, tag="osb", bufs=3,
                                   name="osb")
                    nc.vector.tensor_copy(osb[:], Ops[m][:])
                # full-width recip: custom DVE op mishandles base-64 APs,
                # so run at base 0 over all 128 partitions (cost is
                # free-width bound); only the den half is used, the other
                # half is overwritten by the DMA below.
                rc = att.tile([128, 512], FP32, tag="rc", bufs=2, name="rc")
                nc.vector.reciprocal_approx_fast(out=rc[:], in_=osb[:])
                nc.sync.dma_start(out=rc[hp:hp + 64, :],
                                   in_=rc[dp:dp + 64, :])
                nc.vector.tensor_tensor(oT_sb[hp:hp + 64, i, rsl],
                                        osb[hp:hp + 64, :],
                                        rc[hp:hp + 64, :], op=mult)
        for i in range(QH // 2):
            yield (lambda i=i: headpair(i))

    def p_jobs(r):
        last = (r == NR - 1)

        def po_job(t, nch, split=False):
            tsl = bass.ds(t * 128, 128)
            po = ps512.tile([128, 512], FP32, tag="c", name="po")
            if split:
                # Emit the fh0-2 accumulation now (those oT chunks are
                # ready well before the last head pair's norm lands), and
                # return a closure that finishes fh3 + eviction. Fills
                # the PE while the final norm chain drains.
                for fh in range(3):
                    nc.tensor.matmul(po[:], oT_sb[:, fh, tsl],
                                     wo_sb[:, fh, bass.ts(nch, 512)],
                                     start=(fh == 0), stop=False)

                def finish():
                    nc.tensor.matmul(po[:], oT_sb[:, 3, tsl],
                                     wo_sb[:, 3, bass.ts(nch, 512)],
                                     start=False, stop=True,
                                     skip_group_check=True)
                    _evict(t, nch, po)
                return finish
            for fh in range(4):
                nc.tensor.matmul(po[:], oT_sb[:, fh, tsl],
                                 wo_sb[:, fh, bass.ts(nch, 512)],
                                 start=(fh == 0), stop=(fh == 3))
            _evict(t, nch, po)

        def _evict(t, nch, po):
            tsl = bass.ds(t * 128, 128)
            po_sb = fwork.tile([128, 512], FP32, tag="po", name="po_sb")
            if last:  # ACT is idle after the final exp; spare the DVE queue
                nc.scalar.copy(po_sb[:], po[:])
            else:
                nc.vector.tensor_copy(po_sb[:], po[:])
            nc.sync.dma_start(out=out[tsl, bass.ts(nch, 512)], in_=po_sb[:])

        jobs = [(t, nch) for t in range(4 * r, 4 * r + 4)
                for nch in range(4)]
        if last:
            def first_two():
                f0 = po_job(*jobs[0], split=True)
                f1 = po_job(*jobs[1], split=True)
                f0()
                f1()
            yield first_two
            jobs = jobs[2:]
        for t, nch in jobs:
            yield (lambda t=t, nch=nch: po_job(t, nch))

    # PE warmup: ~4us of throwaway matmuls on already-loaded consts so HAM
    # reaches K=8/8 before the first real chunk, which is DMA-paced.
    mask_rep = bass.AP(tensor=mask_sb.tensor, offset=mask_sb.offset,
                       ap=[mask_sb.ap[0], [0, 4], [1, 128]])
    for w in range(2):
        wps = ps512.tile([128, 512], FP32, tag="c", name="warm")
        for i in range(5):
            nc.tensor.matmul(wps[:], mask_sb[:], mask_rep,
                             start=(i == 0), stop=(i == 4))

    # Sequential rounds: B(r), A(r), then o_proj of the previous round
    # (gives A(r)'s norm chains the next B round to drain). Interleaving
    # B/P jobs into A measured consistently slower — mixing the K=64
    # row-tiled S matmuls with full-array matmuls breaks throughput.
    def ham_filler(n):
        # Round 0's attention is latency-bound; a short burst of throwaway
        # matmuls between its head pairs keeps the PE activity monitor
        # from re-throttling the clock (which would halve B(1)'s rate).
        wps = ps512.tile([128, 512], FP32, tag="c", name="hamf")
        for i in range(n):
            nc.tensor.matmul(wps[:], mask_sb[:], mask_rep,
                             start=(i == 0), stop=(i == n - 1))

    for r in range(NR):
        for job in b_jobs(r):
            job()
        for job in a_jobs(r):
            job()
        if r >= 1:
            for job in p_jobs(r - 1):
                job()
    for job in p_jobs(NR - 1):
        job()

    if dumps is not None:
        for name, sb_tile in (("qT_d", qT_sb), ("kT_d", kT_sb),
                              ("v_d", vext_sb), ("oT_d", oT_sb)):
            if name in dumps:
                nc.sync.dma_start(out=dumps[name][:], in_=sb_tile[:])
    ctx.close()


def _o_proj(nc, ps512, fwork, oT_sb, wo_sb, out, r):
    for t in range(4 * r, 4 * r + 4):
        tsl = bass.ds(t * 128, 128)
        for nch in range(4):
            po = ps512.tile([128, 512], FP32, tag="c", name="po")
            for fh in range(4):
                nc.tensor.matmul(po[:], oT_sb[:, fh, tsl],
                                 wo_sb[:, fh, bass.ts(nch, 512)],
                                 start=(fh == 0), stop=(fh == 3))
            po_sb = fwork.tile([128, 512], FP32, tag="po", name="po_sb")
            nc.vector.tensor_copy(po_sb[:], po[:])
            nc.sync.dma_start(out=out[tsl, bass.ts(nch, 512)], in_=po_sb[:])


def _o_mms(nc, Ops, vext_sb, jv, n_ik, ik, j0, p_sb):
    lsl = bass.ds(j0 * 128, 512 - j0 * 128)
    for m in range(2):
        nc.tensor.matmul(Ops[m][:, lsl],
                         vext_sb[:, ik, jv, bass.ds(64 * (1 - m), 128)],
                         p_sb[:, m, lsl],
                         start=(ik == 0), stop=(ik == n_ik - 1),
                         skip_group_check=(ik != 0))


_NC_CACHE = None


def _get_nc():
    global _NC_CACHE
    if _NC_CACHE is None:
        _NC_CACHE = _build_nc()
    return _NC_CACHE


def _rope_tables(pos):
    """cosT/msinT [128, SEQ] bf16: transposed-layout RoPE tables, head-dim
    pattern duplicated for the two heads stacked per 128 partitions. msinT
    rows 0:32 carry -sin (a-half), 32:64 carry +sin (b-half)."""
    pos = np.asarray(pos, dtype=np.float32)  # [SEQ]
    inv = (1.0 / (np.float32(ROPE_THETA)
                  ** (np.arange(0, HEAD_DIM, 2, dtype=np.float32)
                      / np.float32(HEAD_DIM)))).astype(np.float32)
    fr = pos[:, None] * inv[None, :]                       # [SEQ, 32]
    emb = np.concatenate([fr, fr], axis=-1).astype(np.float32)  # [SEQ, 64]
    cos_t = np.cos(emb).T                                   # [64, SEQ]
    sin_t = np.sin(emb).T
    msin_t = np.concatenate([-sin_t[0:32], sin_t[32:64]], axis=0)
    cosT = np.tile(cos_t, (2, 1)).astype(BF16)
    msinT = np.tile(msin_t, (2, 1)).astype(BF16)
    return np.ascontiguousarray(cosT), np.ascontiguousarray(msinT)


def _make_in_maps(input_ids, Wq, Wk, Wv, Wo, position_ids):
    x = np.asarray(input_ids, dtype=np.float32)
    Wq = np.asarray(Wq, dtype=np.float32)
    Wk = np.asarray(Wk, dtype=np.float32)
    Wv = np.asarray(Wv, dtype=np.float32)
    Wo = np.asarray(Wo, dtype=np.float32)
    pos = np.asarray(position_ids)

    maskt = np.triu(np.ones((128, 128), dtype=np.float32)).astype(BF16)

    in_maps = []
    for c in range(N_CORES):
        b, g = c // TP, c % TP
        xT = np.ascontiguousarray(x[b].T).astype(BF16)
        wq = Wq[:, g * QH * HEAD_DIM:(g + 1) * QH * HEAD_DIM]
        wk = Wk[:, g * KVH * HEAD_DIM:(g + 1) * KVH * HEAD_DIM]
        wv = Wv[:, g * KVH * HEAD_DIM:(g + 1) * KVH * HEAD_DIM]
        wqkv = np.concatenate([wq, wk, wv], axis=1).astype(BF16)
        wo_s = np.ascontiguousarray(
            Wo[g * F_O:(g + 1) * F_O, :]).astype(BF16)
        cosT, msinT = _rope_tables(pos[b])
        in_maps.append({
            "xT": np.ascontiguousarray(xT),
            "wqkv": np.ascontiguousarray(wqkv),
            "wo": wo_s,
            "cosT": cosT,
            "msinT": msinT,
            "maskt": maskt,
        })
    return in_maps


def _run(in_maps, trace=False):
    nc = _get_nc()
    kwargs = {}
    if trace:
        _install_profile_hook()
        kwargs["trace"] = True
    return run_bass_kernel_spmd(nc, in_maps, core_ids=list(range(N_CORES)),
                                **kwargs)


def _install_profile_hook():
    """This image's antenv lacks axon_hooks; register the NTFF profile hook
    manually so trace=True yields hardware exec times."""
    if "antenv.axon_hooks" in sys.modules:
        return
    import antenv
    mod = types.ModuleType("antenv.axon_hooks")
    state = {"hook": None}
    mod.set_axon_ntff_profile_hook = lambda h: state.__setitem__("hook", h)
    mod.get_axon_ntff_profile_hook = lambda: state["hook"]
    sys.modules["antenv.axon_hooks"] = mod
    antenv.axon_hooks = mod
    try:
        from trn_agent_boot.trn_boot import _ntff_profile_via_ctypes
        mod.set_axon_ntff_profile_hook(
            _ntff_profile_via_ctypes("/opt/axon/libaxon_pjrt.so"))
    except Exception:
        pass


def kernel(input_ids, Wq, Wk, Wv, Wo, position_ids):
    in_maps = _make_in_maps(input_ids, Wq, Wk, Wv, Wo, position_ids)
    res = _run(in_maps, trace=bool(os.environ.get("KERNEL_TRACE")))
    if os.environ.get("KERNEL_TRACE"):
        print(f"HW exec time: {res.exec_time_ns} ns "
              f"(mean {res.mean_exec_time_ns})")
    out = np.zeros((BATCH, SEQ, HIDDEN), dtype=np.float32)
    for c in range(N_CORES):
        out[c // TP] += res.results[c]["out"]
    return out


# revision 38
# speedup vs baseline: 1.1360x; 1.1360x over previous
"""Trainium2 Bass kernel for a dense-transformer attention block.

Module: y = o_proj(causal_sdpa(rope(q_proj(x)), rope(k_proj(x)), v_proj(x)))
Shapes: x [2, 2048, 2048], 32 q heads / 8 kv heads, head_dim 64, fp32 I/O.

Sharding (8 NeuronCores): 2-way data parallel over batch x 4-way tensor
parallel over heads. Core c handles batch c//4 and head group c%4
(8 q heads, 2 kv heads). Each core produces a partial [2048, 2048]
output (its heads' slice of o_proj); the host sums the 4 partials per
batch.

On-device layout (v2 — transpose-free):
- q/k projections run weight-stationary (lhsT = W chunk, moving = xT), so
  they produce qT/kT [feat, seq] directly — no PE transposes. RoPE is
  applied at PSUM eviction in this transposed layout: rotate-half
  partners are +-32 partitions away, handled by four [32,512] shift-mult
  ops against a sign-folded sin table, plus one cos-mult and one add.
- v runs x-stationary (natural [seq, feat] layout) and is packed into
  vext = [V | ones] 128-wide stationary tiles.
- Scores are computed transposed (ST = K Q^T per 128x512 block,
  64-partition contraction, no zero padding), exp on eviction with
  scale=1/8 (no max subtraction; |S/8| < ~10 for this distribution).
- O matmul is V-stationary: out = vext^T @ P gives OT [d, q] on
  partitions 0:64 and the softmax denominator replicated on partitions
  64:128 (the 64 ones-columns broadcast it). Normalization is then a
  pure elementwise reciprocal+multiply writing oT [feat, seq] tiles,
  which feed o_proj's stationary operand directly — no O transposes.
- k is written duplicated into both 64-partition halves so the S matmul
  stationary base partition always matches the q head's base partition.
- Work is interleaved in rounds over 512-wide seq chunks: projection
  round B(r) then attention round A(r) (which needs only k/v tiles
  0..4r+3), so exp on the ACT engine overlaps PE work from early on and
  input DMA is pipelined with the first matmuls.
"""

import os
import sys
import types

import numpy as np

sys.path.insert(0, "/opt/trn_rl_repo")

import concourse.bacc as bacc  # noqa: E402
import concourse.bass as bass  # noqa: E402
import concourse.tile as tile  # noqa: E402
from concourse import mybir  # noqa: E402
from concourse.bass_utils import run_bass_kernel_spmd  # noqa: E402

try:
    import ml_dtypes
    BF16 = ml_dtypes.bfloat16
except ImportError:  # pragma: no cover
    BF16 = np.dtype("bfloat16")

HIDDEN = 2048
SEQ = 2048
BATCH = 2
N_HEADS = 32
N_KV_HEADS = 8
HEAD_DIM = 64
ROPE_THETA = 10000.0

N_CORES = 8
TP = 4                      # head-parallel ways
QH = N_HEADS // TP          # 8 q heads per core
KVH = N_KV_HEADS // TP      # 2 kv heads per core
KT = HIDDEN // 128          # 16 contraction tiles
TT = SEQ // 128             # 16 seq tiles
NR = 4                      # rounds (512-wide seq chunks)
F_O = QH * HEAD_DIM         # 512

FP32 = mybir.dt.float32
BF16_DT = mybir.dt.bfloat16
F8_DT = mybir.dt.float8e4
DR = mybir.MatmulPerfMode.DoubleRow
KB = 8                      # fp8 DoubleRow contraction blocks (256 wide)
W_SCALE = 64.0              # host upscale of W into fp8 normal range

try:
    F8NP = ml_dtypes.float8_e4m3fn
except Exception:  # pragma: no cover
    F8NP = None


def _build_nc():
    nc = bacc.Bacc("TRN2", target_bir_lowering=False, debug=False)

    xT = nc.dram_tensor("xT", [HIDDEN, SEQ], BF16_DT, kind="ExternalInput")
    wqkv = nc.dram_tensor("wqkv", [HIDDEN, 768], BF16_DT, kind="ExternalInput")
    wo = nc.dram_tensor("wo", [F_O, HIDDEN], BF16_DT, kind="ExternalInput")
    cosT = nc.dram_tensor("cosT", [128, SEQ], BF16_DT, kind="ExternalInput")
    msinT = nc.dram_tensor("msinT", [128, SEQ], BF16_DT, kind="ExternalInput")
    maskt = nc.dram_tensor("maskt", [128, 128], BF16_DT, kind="ExternalInput")
    out = nc.dram_tensor("out", [SEQ, HIDDEN], FP32, kind="ExternalOutput")

    with tile.TileContext(nc) as tc:
        _emit(nc, tc, xT, wqkv, wo, cosT, msinT, maskt, out)
    nc.compile()
    return nc


def _emit(nc, tc, xT, wqkv, wo, cosT, msinT, maskt, out, dumps=None):
    from contextlib import ExitStack
    ctx = ExitStack()
    Exp = mybir.ActivationFunctionType.Exp
    mult = mybir.AluOpType.mult

    const = ctx.enter_context(tc.tile_pool(name="const", bufs=1))
    persist = ctx.enter_context(tc.tile_pool(name="persist", bufs=1))
    big = ctx.enter_context(tc.tile_pool(name="big", bufs=1))
    bwork = ctx.enter_context(tc.tile_pool(name="bwork", bufs=4))
    att = ctx.enter_context(tc.tile_pool(name="att", bufs=1))
    fwork = ctx.enter_context(tc.tile_pool(name="fwork", bufs=3))
    psS = ctx.enter_context(tc.tile_pool(name="psS", bufs=2, space="PSUM"))
    psO = ctx.enter_context(tc.tile_pool(name="psO", bufs=2, space="PSUM"))
    ps512 = ctx.enter_context(tc.tile_pool(name="ps512", bufs=2, space="PSUM"))

    # ---- constants / persistent buffers ----
    mask_sb = const.tile([128, 128], BF16_DT)
    cos_sb = const.tile([128, SEQ], BF16_DT)
    msin_sb = const.tile([128, SEQ], BF16_DT)
    wo_sb = const.tile([128, 4, HIDDEN], BF16_DT)

    # qT: head h lives at partitions 64*(h%2), pair index h//2.
    # kT: kv head j duplicated on partitions 0:64 AND 64:128 so the S
    # matmul's stationary base always matches the q head's base partition.
    # vext: two variants per (seq tile, kv head): [V | ones] for even
    # heads, [ones | V] for odd heads, so OT lands on the head's oT
    # partitions and the ones block broadcasts the softmax denominator to
    # the other 64 partitions.
    # oT: o_proj feature chunk fh holds heads 2fh (parts 0:64), 2fh+1.
    qT_sb = persist.tile([128, NR, SEQ], BF16_DT, name="qT")
    kT_sb = persist.tile([128, KVH, SEQ], BF16_DT, name="kT")
    vext_sb = persist.tile([128, TT, KVH, 192], BF16_DT, name="vext")
    oT_sb = persist.tile([128, 4, SEQ], BF16_DT, name="oT")
    xT_sb = big.tile([128, KT, SEQ], BF16_DT)
    w_sb = big.tile([128, KT, 768], BF16_DT)

    nc.gpsimd.memset(vext_sb[:, :, :, 0:64], 1.0)
    nc.gpsimd.memset(vext_sb[:, :, :, 128:192], 1.0)

    # ---- input DMA: consts, then (w, x) per k-slice for round 0, then
    # the remaining x seq-chunks ----
    nc.sync.dma_start(out=mask_sb[:], in_=maskt[:])
    nc.sync.dma_start(out=cos_sb[:], in_=cosT[:])
    nc.sync.dma_start(out=msin_sb[:], in_=msinT[:])
    xT_r = xT[:].rearrange("(k p) t -> p k t", p=128)
    w_r = wqkv[:].rearrange("(k p) f -> p k f", p=128)
    for k in range(KT):
        nc.sync.dma_start(out=w_sb[:, k, :], in_=w_r[:, k, :])
        nc.sync.dma_start(out=xT_sb[:, k, 0:512], in_=xT_r[:, k, 0:512])
    nc.sync.dma_start(out=wo_sb[:], in_=wo[:].rearrange("(k p) d -> p k d", p=128))
    for k in range(KT):
        nc.sync.dma_start(out=xT_sb[:, k, 512:SEQ], in_=xT_r[:, k, 512:SEQ])

    def rope_evict(ps, dst0, rsl):
        """RoPE at PSUM eviction, transposed layout. ps is [128, 512] with
        two heads stacked (64 partitions each). The rotate-half partner
        shuffle (+-32 partitions) runs as SBUF->SBUF DMAs since DVE lanes
        cannot cross partitions; the multiplies are then full-width and
        partition-aligned."""
        src = bwork.tile([128, 512], BF16_DT, tag="src", name="src")
        nc.scalar.copy(src[:], ps)
        shf = bwork.tile([128, 512], BF16_DT, tag="shf", name="shf")
        for od, os_ in ((0, 32), (32, 0), (64, 96), (96, 64)):
            nc.gpsimd.dma_start(out=shf[od:od + 32, :],
                                in_=src[os_:os_ + 32, :])
        nc.vector.tensor_tensor(dst0, src[:], cos_sb[:, rsl], op=mult)
        tmp = bwork.tile([128, 512], BF16_DT, tag="tmp", name="tmp")
        nc.vector.tensor_tensor(tmp[:], shf[:], msin_sb[:, rsl], op=mult)
        nc.vector.tensor_add(dst0, dst0, tmp[:])

    # ---- job generators: B(r) projection jobs, A(r) head jobs,
    # P(r) o_proj jobs. Emission interleaves streams job-by-job so the PE
    # queue always has independent work to fill dependency bubbles (keeps
    # HAM from re-throttling on >3us gaps). ----

    def b_jobs(r):
        rsl = bass.ds(r * 512, 512)

        def kchunk():
            kps = ps512.tile([128, 512], FP32, tag="c", name="kps")
            for k in range(KT):
                nc.tensor.matmul(kps[:], w_sb[:, k, 512:640],
                                 xT_sb[:, k, rsl],
                                 start=(k == 0), stop=(k == KT - 1))
            # RoPE into a temp, then duplicate each kv head to both halves
            # (aligned halves via DVE, cross-partition halves via DMA).
            kt_tmp = bwork.tile([128, 512], BF16_DT, tag="ktt", name="kt_tmp")
            rope_evict(kps[:], kt_tmp[:], rsl)
            nc.vector.tensor_copy(kT_sb[0:64, 0, rsl], kt_tmp[0:64, :])
            nc.gpsimd.dma_start(out=kT_sb[64:128, 0, rsl],
                                in_=kt_tmp[0:64, :])
            nc.gpsimd.dma_start(out=kT_sb[0:64, 1, rsl],
                                in_=kt_tmp[64:128, :])
            nc.vector.tensor_copy(kT_sb[64:128, 1, rsl], kt_tmp[64:128, :])
        yield kchunk

        def vtile(t):
            tsl = bass.ds(t * 128, 128)
            vps = ps512.tile([128, 512], FP32, tag="c", name="vps")
            for k in range(KT):
                nc.tensor.matmul(vps[:, 0:128], xT_sb[:, k, tsl],
                                 w_sb[:, k, 640:768],
                                 start=(k == 0), stop=(k == KT - 1))
            vnat = vps[:, 0:128].rearrange("p (j d) -> p j d", j=KVH)
            nc.vector.tensor_copy(vext_sb[:, t, :, 64:128], vnat)

        def qchunk(fc):
            qps = ps512.tile([128, 512], FP32, tag="c", name="qps")
            for k in range(KT):
                nc.tensor.matmul(qps[:], w_sb[:, k, bass.ds(fc * 128, 128)],
                                 xT_sb[:, k, rsl],
                                 start=(k == 0), stop=(k == KT - 1))
            rope_evict(qps[:], qT_sb[:, fc, rsl], rsl)

        # q chunks before v tiles: the q/k RoPE eviction chains (DVE +
        # shuffle DMA) are long, the v evictions short, so this order
        # has all chains drained by the time A(r) starts.
        for fc in range(4):
            yield (lambda fc=fc: qchunk(fc))
        for t in range(4 * r, 4 * r + 4):
            yield (lambda t=t: vtile(t))

    def a_jobs(r):
        rsl = bass.ds(r * 512, 512)
        n_ik = 4 * r + 4

        def headpair(i):
            # heads h0=2i (partitions 0:64) and h1=2i+1 (64:128). Their S
            # matmuls contract only 64 rows each, at row groups 0/64 —
            # the PE runs them concurrently (per-subarray row tiling), so
            # the pair's score block costs one matmul's wall time.
            jv = (2 * i) // (QH // KVH)
            Ops = [psO.tile([128, 512], FP32, tag="O", name=f"Ops{r}_{i}{m}")
                   for m in range(2)]
            pend = []  # [(ik, j0, p_sb), ...] awaiting O matmuls
            for ik in range(n_ik):
                j0 = max(0, ik - 4 * r)
                lsl = bass.ds(j0 * 128, 512 - j0 * 128)
                qsl = bass.ds(r * 512 + j0 * 128, 512 - j0 * 128)
                stp = psS.tile([128, 2, 512], FP32, tag="st", name="stp")
                for m, hp in ((0, 0), (1, 64)):
                    nc.tensor.matmul(stp[:, m, lsl],
                                     kT_sb[hp:hp + 64, jv, bass.ts(ik, 128)],
                                     qT_sb[hp:hp + 64, i, qsl],
                                     start=True, stop=True)
                p_sb = att.tile([128, 2, 512], BF16_DT, tag="p", bufs=4,
                                name="p_sb")
                nc.scalar.activation(p_sb[:, :, lsl], stp[:, :, lsl],
                                     Exp, scale=0.125)
                if ik >= 4 * r:  # diagonal tile: apply causal mask
                    for m in range(2):
                        nc.gpsimd.tensor_mul(
                            p_sb[:, m, bass.ts(j0, 128)],
                            p_sb[:, m, bass.ts(j0, 128)], mask_sb[:])
                if len(pend) >= 1:
                    _o_mms(nc, Ops, vext_sb, jv, n_ik, *pend.pop(0))
                pend.append((ik, j0, p_sb))
            for args in pend:
                _o_mms(nc, Ops, vext_sb, jv, n_ik, *args)

            # Evict both O accumulators to SBUF right away so the PSUM
            # banks free for the next pair; normalization then runs off
            # the critical path. OT sits at partitions hp:hp+64, the
            # replicated denominator at the other half; a small
            # SBUF->SBUF DMA moves the reciprocal to OT's partitions so
            # the normalizing multiply is partition-aligned.
            direct = (r == NR - 1 and i == QH // 2 - 1)
            for m, hp in ((0, 0), (1, 64)):
                dp = 64 - hp
                if direct:
                    osb = Ops[m]
                else:
                    osb = att.tile([128, 512], FP32, tag="osb", bufs=3,
                                   name="osb")
                    nc.vector.tensor_copy(osb[:], Ops[m][:])
                # full-width recip: custom DVE op mishandles base-64 APs,
                # so run at base 0 over all 128 partitions (cost is
                # free-width bound); only the den half is used, the other
                # half is overwritten by the DMA below.
                rc = att.tile([128, 512], FP32, tag="rc", bufs=2, name="rc")
                nc.vector.reciprocal_approx_fast(out=rc[:], in_=osb[:])
                nc.sync.dma_start(out=rc[hp:hp + 64, :],
                                   in_=rc[dp:dp + 64, :])
                nc.vector.tensor_tensor(oT_sb[hp:hp + 64, i, rsl],
                                        osb[hp:hp + 64, :],
                                        rc[hp:hp + 64, :], op=mult)
        for i in range(QH // 2):
            yield (lambda i=i: headpair(i))

    def p_jobs(r):
        last = (r == NR - 1)

        def po_job(t, nch, split=False):
            tsl = bass.ds(t * 128, 128)
            po = ps512.tile([128, 512], FP32, tag="c", name="po")
            if split:
                # Emit the fh0-2 accumulation now (those oT chunks are
                # ready well before the last head pair's norm lands), and
                # return a closure that finishes fh3 + eviction. Fills
                # the PE while the final norm chain drains.
                for fh in range(3):
                    nc.tensor.matmul(po[:], oT_sb[:, fh, tsl],
                                     wo_sb[:, fh, bass.ts(nch, 512)],
                                     start=(fh == 0), stop=False)

                def finish():
                    nc.tensor.matmul(po[:], oT_sb[:, 3, tsl],
                                     wo_sb[:, 3, bass.ts(nch, 512)],
                                     start=False, stop=True,
                                     skip_group_check=True)
                    _evict(t, nch, po)
                return finish
            for fh in range(4):
                nc.tensor.matmul(po[:], oT_sb[:, fh, tsl],
                                 wo_sb[:, fh, bass.ts(nch, 512)],
                                 start=(fh == 0), stop=(fh == 3))
            _evict(t, nch, po)

        def _evict(t, nch, po):
            tsl = bass.ds(t * 128, 128)
            po_sb = fwork.tile([128, 512], FP32, tag="po", name="po_sb")
            if last:  # ACT is idle after the final exp; spare the DVE queue
                nc.scalar.copy(po_sb[:], po[:])
            else:
                nc.vector.tensor_copy(po_sb[:], po[:])
            nc.sync.dma_start(out=out[tsl, bass.ts(nch, 512)], in_=po_sb[:])

        jobs = [(t, nch) for t in range(4 * r, 4 * r + 4)
                for nch in range(4)]
        if last:
            def first_two():
                f0 = po_job(*jobs[0], split=True)
                f1 = po_job(*jobs[1], split=True)
                f0()
                f1()
            yield first_two
            jobs = jobs[2:]
        for t, nch in jobs:
            yield (lambda t=t, nch=nch: po_job(t, nch))

    # PE warmup: ~4us of throwaway matmuls on already-loaded consts so HAM
    # reaches K=8/8 before the first real chunk, which is DMA-paced.
    mask_rep = bass.AP(tensor=mask_sb.tensor, offset=mask_sb.offset,
                       ap=[mask_sb.ap[0], [0, 4], [1, 128]])
    for w in range(2):
        wps = ps512.tile([128, 512], FP32, tag="c", name="warm")
        for i in range(5):
            nc.tensor.matmul(wps[:], mask_sb[:], mask_rep,
                             start=(i == 0), stop=(i == 4))

    # Sequential rounds: B(r), A(r), then o_proj of the previous round
    # (gives A(r)'s norm chains the next B round to drain). Interleaving
    # B/P jobs into A measured consistently slower — mixing the K=64
    # row-tiled S matmuls with full-array matmuls breaks throughput.
    def ham_filler(n):
        # Round 0's attention is latency-bound; a short burst of throwaway
        # matmuls between its head pairs keeps the PE activity monitor
        # from re-throttling the clock (which would halve B(1)'s rate).
        wps = ps512.tile([128, 512], FP32, tag="c", name="hamf")
        for i in range(n):
            nc.tensor.matmul(wps[:], mask_sb[:], mask_rep,
                             start=(i == 0), stop=(i == n - 1))

    for r in range(NR):
        for job in b_jobs(r):
            job()
        pj = list(p_jobs(r - 1)) if r >= 1 else []
        for n, job in enumerate(a_jobs(r)):
            job()
            # A rounds are ACT(exp)-bound; slot the previous round's
            # o_proj matmuls into the PE slack between head pairs.
            take, pj = pj[:4], pj[4:]
            for p in take:
                p()
        for p in pj:
            p()
    for job in p_jobs(NR - 1):
        job()

    if dumps is not None:
        for name, sb_tile in (("qT_d", qT_sb), ("kT_d", kT_sb),
                              ("v_d", vext_sb), ("oT_d", oT_sb)):
            if name in dumps:
                nc.sync.dma_start(out=dumps[name][:], in_=sb_tile[:])
    ctx.close()


def _o_proj(nc, ps512, fwork, oT_sb, wo_sb, out, r):
    for t in range(4 * r, 4 * r + 4):
        tsl = bass.ds(t * 128, 128)
        for nch in range(4):
            po = ps512.tile([128, 512], FP32, tag="c", name="po")
            for fh in range(4):
                nc.tensor.matmul(po[:], oT_sb[:, fh, tsl],
                                 wo_sb[:, fh, bass.ts(nch, 512)],
                                 start=(fh == 0), stop=(fh == 3))
            po_sb = fwork.tile([128, 512], FP32, tag="po", name="po_sb")
            nc.vector.tensor_copy(po_sb[:], po[:])
            nc.sync.dma_start(out=out[tsl, bass.ts(nch, 512)], in_=po_sb[:])


def _o_mms(nc, Ops, vext_sb, jv, n_ik, ik, j0, p_sb):
    lsl = bass.ds(j0 * 128, 512 - j0 * 128)
    for m in range(2):
        nc.tensor.matmul(Ops[m][:, lsl],
                         vext_sb[:, ik, jv, bass.ds(64 * (1 - m), 128)],
                         p_sb[:, m, lsl],
                         start=(ik == 0), stop=(ik == n_ik - 1),
                         skip_group_check=(ik != 0))


_NC_CACHE = None


def _get_nc():
    global _NC_CACHE
    if _NC_CACHE is None:
        _NC_CACHE = _build_nc()
    return _NC_CACHE


def _rope_tables(pos):
    """cosT/msinT [128, SEQ] bf16: transposed-layout RoPE tables, head-dim
    pattern duplicated for the two heads stacked per 128 partitions. msinT
    rows 0:32 carry -sin (a-half), 32:64 carry +sin (b-half)."""
    pos = np.asarray(pos, dtype=np.float32)  # [SEQ]
    inv = (1.0 / (np.float32(ROPE_THETA)
                  ** (np.arange(0, HEAD_DIM, 2, dtype=np.float32)
                      / np.float32(HEAD_DIM)))).astype(np.float32)
    fr = pos[:, None] * inv[None, :]                       # [SEQ, 32]
    emb = np.concatenate([fr, fr], axis=-1).astype(np.float32)  # [SEQ, 64]
    cos_t = np.cos(emb).T                                   # [64, SEQ]
    sin_t = np.sin(emb).T
    msin_t = np.concatenate([-sin_t[0:32], sin_t[32:64]], axis=0)
    cosT = np.tile(cos_t, (2, 1)).astype(BF16)
    msinT = np.tile(msin_t, (2, 1)).astype(BF16)
    return np.ascontiguousarray(cosT), np.ascontiguousarray(msinT)


def _make_in_maps(input_ids, Wq, Wk, Wv, Wo, position_ids):
    x = np.asarray(input_ids, dtype=np.float32)
    Wq = np.asarray(Wq, dtype=np.float32)
    Wk = np.asarray(Wk, dtype=np.float32)
    Wv = np.asarray(Wv, dtype=np.float32)
    Wo = np.asarray(Wo, dtype=np.float32)
    pos = np.asarray(position_ids)

    maskt = np.triu(np.ones((128, 128), dtype=np.float32)).astype(BF16)

    in_maps = []
    for c in range(N_CORES):
        b, g = c // TP, c % TP
        xT = np.ascontiguousarray(x[b].T).astype(BF16)
        wq = Wq[:, g * QH * HEAD_DIM:(g + 1) * QH * HEAD_DIM]
        wk = Wk[:, g * KVH * HEAD_DIM:(g + 1) * KVH * HEAD_DIM]
        wv = Wv[:, g * KVH * HEAD_DIM:(g + 1) * KVH * HEAD_DIM]
        wqkv = np.concatenate([wq, wk, wv], axis=1).astype(BF16)
        wo_s = np.ascontiguousarray(
            Wo[g * F_O:(g + 1) * F_O, :]).astype(BF16)
        cosT, msinT = _rope_tables(pos[b])
        in_maps.append({
            "xT": np.ascontiguousarray(xT),
            "wqkv": np.ascontiguousarray(wqkv),
            "wo": wo_s,
            "cosT": cosT,
            "msinT": msinT,
            "maskt": maskt,
        })
    return in_maps


def _run(in_maps, trace=False):
    nc = _get_nc()
    kwargs = {}
    if trace:
        _install_profile_hook()
        kwargs["trace"] = True
    return run_bass_kernel_spmd(nc, in_maps, core_ids=list(range(N_CORES)),
                                **kwargs)


def _install_profile_hook():
    """This image's antenv lacks axon_hooks; register the NTFF profile hook
    manually so trace=True yields hardware exec times."""
    if "antenv.axon_hooks" in sys.modules:
        return
    import antenv
    mod = types.ModuleType("antenv.axon_hooks")
    state = {"hook": None}
    mod.set_axon_ntff_profile_hook = lambda h: state.__setitem__("hook", h)
    mod.get_axon_ntff_profile_hook = lambda: state["hook"]
    sys.modules["antenv.axon_hooks"] = mod
    antenv.axon_hooks = mod
    try:
        from trn_agent_boot.trn_boot import _ntff_profile_via_ctypes
        mod.set_axon_ntff_profile_hook(
            _ntff_profile_via_ctypes("/opt/axon/libaxon_pjrt.so"))
    except Exception:
        pass


def kernel(input_ids, Wq, Wk, Wv, Wo, position_ids):
    in_maps = _make_in_maps(input_ids, Wq, Wk, Wv, Wo, position_ids)
    res = _run(in_maps, trace=bool(os.environ.get("KERNEL_TRACE")))
    if os.environ.get("KERNEL_TRACE"):
        print(f"HW exec time: {res.exec_time_ns} ns "
              f"(mean {res.mean_exec_time_ns})")
    out = np.zeros((BATCH, SEQ, HIDDEN), dtype=np.float32)
    for c in range(N_CORES):
        out[c // TP] += res.results[c]["out"]
    return out
